# revision 1
# baseline (speedup 1.0000x reference)
"""Trainium2 Bass kernel for nn_EquivairantMultiheadAttention (sparse attention).

Contract: kernel(**inputs) takes the FULL unsharded numpy inputs (as produced by
setup_inputs()) and returns the FULL (B, N, COUT) float32 output.

Sharding: 8 cores = data-parallel over batch (2) x sequence-parallel over the
query dim n (4 slices of 512). Each core receives its batch's coset_functions
(transposed), its query-slice of pairwise_g / nbhd_idx, and all weights.

Math notes (equivalences used, all exact):
 - t3 = einsum(K, u) and every other per-(n,h) constant (b_k.Q, b_l.(Q+v))
   is constant across the softmax axis m, hence drops out of softmax.
 - e . (Q+v) = pg . G with G[n,h,:] = W_l[h-block,:]^T (Q+v)[n,h-block]
   (the b_l part is m-constant -> dropped).
 - v_lin = nbhd_cf @ W_in^T + b_in: aggregate V rows (V = coset @ W_in^T) and
   add b_in after the softmax-weighted sum (weights sum to 1).
 - mask is all ones (spec: fill=ones) -> masking is a no-op.

Device gathers use gpsimd.dma_gather (int16 idx, <=NIC idxs/call,
elem=256B-multiple). pairwise_g rows (24B) are gathered as 8-row packs of
256B; the wanted row is selected with predicated copies using the low 3 bits
of the neighbor index.
"""

import math
import sys

import numpy as np

sys.path.insert(0, "/opt/trn_rl_repo")

B, N, M = 2, 2048, 64
C = 256  # CIN == COUT
H, D, POS = 8, 32, 6
NQ = 512  # queries per core
QB = 4  # query blocks of 128 per core
P = 128
NCORES = 8
INV_SQRT_D = 1.0 / math.sqrt(D)
NIC = 1024  # max idxs per dma_gather call (HW: larger crashes the exec unit)
MH = M // 2  # m-half

_compiled = {}


def build_bass():
    import concourse.bacc as bacc
    import concourse.mybir as mybir
    import concourse.tile as tile
    from concourse.masks import make_identity

    dt = mybir.dt
    nc = bacc.Bacc("TRN2", target_bir_lowering=False, debug=False,
                   enable_asserts=False, num_devices=NCORES)

    f32 = dt.float32
    i16 = dt.int16

    # ---- DRAM inputs (per core) ----
    d_cosetT = nc.dram_tensor("cosetT", [2, P, N], f32, kind="ExternalInput")
    d_cosetQT = nc.dram_tensor("cosetQT", [2, P, NQ], f32, kind="ExternalInput")
    d_wqT = nc.dram_tensor("wqT", [2, P, C], f32, kind="ExternalInput")
    d_wkT = nc.dram_tensor("wkT", [2, P, C], f32, kind="ExternalInput")
    d_winT = nc.dram_tensor("winT", [2, P, C], f32, kind="ExternalInput")
    d_woT = nc.dram_tensor("woT", [2, P, C], f32, kind="ExternalInput")
    d_wlBD = nc.dram_tensor("wlBD", [2, P, H * POS], f32, kind="ExternalInput")
    d_bqv = nc.dram_tensor("bqv", [2, P, 1], f32, kind="ExternalInput")
    d_bqmat = nc.dram_tensor("bqmat", [P, C], f32, kind="ExternalInput")
    d_binmat = nc.dram_tensor("binmat", [P, C], f32, kind="ExternalInput")
    d_boutmat = nc.dram_tensor("boutmat", [P, C], f32, kind="ExternalInput")
    # wrapped int16 index lists (m-major: list pos i = m*128 + n_sub)
    d_idxw = nc.dram_tensor("idxw", [P, QB, M * P // 16], i16,
                            kind="ExternalInput")
    d_pgidxw = nc.dram_tensor("pgidxw", [P, QB, M * P // 16], i16,
                              kind="ExternalInput")
    # parity masks: pgmask[k-1][p, qb, m] = 1.0 if (flatidx & 7) == k
    d_pgmask = nc.dram_tensor("pgmask", [P, 7, QB, M], dt.uint8,
                              kind="ExternalInput")
    # pairwise_g packed: row r holds flat rows 8r..8r+7, each padded 6->8
    d_pgpack = nc.dram_tensor("pgpack", [NQ * N // 8, 64], f32,
                              kind="ExternalInput")
    d_out = nc.dram_tensor("out", [QB, P, C], f32, kind="ExternalOutput")

    add = mybir.AluOpType.add
    mult = mybir.AluOpType.mult

    with tile.TileContext(nc) as tc:
        with (
            tc.tile_pool(name="const", bufs=1) as constp,
            tc.tile_pool(name="dram", bufs=1, space="DRAM") as dramp,
            tc.tile_pool(name="psum", bufs=2, space="PSUM") as psump,
            tc.tile_pool(name="evac", bufs=2) as evacp,
        ):
            ident = constp.tile([P, P], f32)
            make_identity(nc, ident[:])

            wqT = constp.tile([P, 2, C], f32)
            wkT = constp.tile([P, 2, C], f32)
            winT = constp.tile([P, 2, C], f32)
            woT = constp.tile([P, 2, C], f32)
            wlBD = constp.tile([P, 2, H * POS], f32)
            for cc in range(2):
                nc.sync.dma_start(wqT[:, cc, :], d_wqT[cc])
                nc.sync.dma_start(wkT[:, cc, :], d_wkT[cc])
                nc.sync.dma_start(winT[:, cc, :], d_winT[cc])
                nc.sync.dma_start(woT[:, cc, :], d_woT[cc])
                nc.sync.dma_start(wlBD[:, cc, :], d_wlBD[cc])
            bqv = constp.tile([P, 2, 1], f32)
            nc.sync.dma_start(bqv[:], d_bqv.ap().rearrange("c p one -> p c one"))
            bqmat = constp.tile([P, C], f32)
            binmat = constp.tile([P, C], f32)
            boutmat = constp.tile([P, C], f32)
            nc.sync.dma_start(bqmat[:], d_bqmat.ap())
            nc.sync.dma_start(binmat[:], d_binmat.ap())
            nc.sync.dma_start(boutmat[:], d_boutmat.ap())
            idxw = constp.tile([P, QB, M * P // 16], i16)
            pgidxw = constp.tile([P, QB, M * P // 16], i16)
            pgmask = constp.tile([P, 7, QB, M], dt.uint8)
            nc.sync.dma_start(idxw[:], d_idxw.ap())
            nc.sync.dma_start(pgidxw[:], d_pgidxw.ap())
            nc.sync.dma_start(pgmask[:], d_pgmask.ap())

            # K,V rows in DRAM scratch
            kdr = dramp.tile([N, C], f32)
            vdr = dramp.tile([N, C], f32)

            q_rows = constp.tile([P, QB, C], f32)
            g_rows = constp.tile([P, QB, H * POS], f32)

            with tc.tile_pool(name="prep", bufs=1) as prepp:
                cosT = prepp.tile([P, 2, N], f32)
                cosQT = prepp.tile([P, 2, NQ], f32)
                for cc in range(2):
                    nc.sync.dma_start(cosT[:, cc, :], d_cosetT[cc])
                    nc.sync.dma_start(cosQT[:, cc, :], d_cosetQT[cc])

                for jt in range(N // P):
                    ps = psump.tile([P, 2 * C], f32, tag="ps")
                    for cc in range(2):
                        nc.tensor.matmul(ps[:, 0:C],
                                         lhsT=cosT[:, cc, jt * P:(jt + 1) * P],
                                         rhs=wkT[:, cc, :],
                                         start=(cc == 0), stop=(cc == 1))
                    for cc in range(2):
                        nc.tensor.matmul(ps[:, C:2 * C],
                                         lhsT=cosT[:, cc, jt * P:(jt + 1) * P],
                                         rhs=winT[:, cc, :],
                                         start=(cc == 0), stop=(cc == 1))
                    kv_sb = evacp.tile([P, 2 * C], f32, tag="kvevac")
                    nc.vector.tensor_copy(kv_sb[:], ps[:])
                    nc.sync.dma_start(kdr[jt * P:(jt + 1) * P, :],
                                      kv_sb[:, 0:C])
                    nc.sync.dma_start(vdr[jt * P:(jt + 1) * P, :],
                                      kv_sb[:, C:2 * C])

                for nt in range(QB):
                    ps = psump.tile([P, C], f32, tag="ps")
                    for cc in range(2):
                        nc.tensor.matmul(ps[:],
                                         lhsT=cosQT[:, cc, nt * P:(nt + 1) * P],
                                         rhs=wqT[:, cc, :],
                                         start=(cc == 0), stop=(cc == 1))
                    nc.vector.tensor_tensor(out=q_rows[:, nt, :], in0=ps[:],
                                            in1=bqmat[:], op=add)

                qvT = prepp.tile([P, 2, NQ], f32)
                for cc2 in range(2):
                    ps = psump.tile([P, NQ], f32, tag="ps")
                    for cc in range(2):
                        nc.tensor.matmul(ps[:],
                                         lhsT=wqT[:, cc, cc2 * P:(cc2 + 1) * P],
                                         rhs=cosQT[:, cc, :],
                                         start=(cc == 0), stop=(cc == 1))
                    nc.vector.tensor_tensor(
                        out=qvT[:, cc2, :], in0=ps[:],
                        in1=bqv[:, cc2, :].broadcast_to([P, NQ]), op=add)
                for nt in range(QB):
                    ps = psump.tile([P, H * POS], f32, tag="ps")
                    for cc in range(2):
                        nc.tensor.matmul(ps[:],
                                         lhsT=qvT[:, cc, nt * P:(nt + 1) * P],
                                         rhs=wlBD[:, cc, :],
                                         start=(cc == 0), stop=(cc == 1))
                    nc.vector.tensor_copy(g_rows[:, nt, :], ps[:])

            # ---- main loop: per query block, per m-half ----
            with (
                tc.tile_pool(name="gath", bufs=3) as gathp,
                tc.tile_pool(name="small", bufs=2) as smallp,
                tc.tile_pool(name="t2p", bufs=1) as t2p,
            ):
              NH = MH * P  # idxs per m-half = 4096
              for qb in range(QB):
                A = smallp.tile([P, H, M], f32, tag="A")
                pgg = t2p.tile([P, M, 64], f32, tag="pgg")
                for k in range(M * P // NIC):  # pg gather chunks
                    nc.gpsimd.dma_gather(
                        out_ap=pgg[:, k * (NIC // P):(k + 1) * (NIC // P), :],
                        in_ap=d_pgpack.ap()[qb * (P * N // 8):
                                            (qb + 1) * (P * N // 8), :],
                        idxs_ap=idxw_slice(pgidxw, qb, k, NIC),
                        num_idxs=NIC, num_idxs_reg=NIC, elem_size=64)
                # parity select: pg6[p, m, :] = pgg[p, m, par*8 : par*8+6]
                pg6 = t2p.tile([P, M, 8], f32, tag="pg6")
                nc.vector.tensor_copy(pg6[:, :, 0:POS], pgg[:, :, 0:POS])
                for k in range(1, 8):
                    nc.vector.copy_predicated(
                        out=pg6[:, :, 0:POS],
                        mask=pgmask[:, k - 1, qb, :][:, :, None]
                            .broadcast_to([P, M, POS]),
                        data=pgg[:, :, 8 * k:8 * k + POS])
                # t2 -> A (strided out: A[p, h, m]), in two m-halves
                for mh in range(2):
                    t2t = t2p.tile([P, MH, H, POS], f32, tag="t2t")
                    nc.vector.tensor_tensor(
                        out=t2t[:],
                        in0=pg6[:, mh * MH:(mh + 1) * MH, 0:POS][:, :, None, :]
                            .broadcast_to([P, MH, H, POS]),
                        in1=g_rows[:, qb, :]
                            .rearrange("p (h pp) -> p h pp", pp=POS)
                            [:, None, :, :].broadcast_to([P, MH, H, POS]),
                        op=mult)
                    nc.vector.tensor_reduce(
                        out=A[:, :, mh * MH:(mh + 1) * MH].transpose([0, 2, 1]),
                        in_=t2t[:], axis=mybir.AxisListType.X, op=add)

                # scores per m-half
                for mh in range(2):
                    kg = gathp.tile([P, MH, C], f32, tag="gbuf")
                    for k in range(NH // NIC):
                        nc.gpsimd.dma_gather(
                            out_ap=kg[:, k * (NIC // P):(k + 1) * (NIC // P), :],
                            in_ap=kdr[:],
                            idxs_ap=idxw_slice(idxw, qb, mh * (NH // NIC) + k,
                                               NIC),
                            num_idxs=NIC, num_idxs_reg=NIC, elem_size=C)
                    nc.vector.tensor_tensor(
                        out=kg[:], in0=kg[:],
                        in1=q_rows[:, qb, :][:, None, :]
                            .broadcast_to([P, MH, C]),
                        op=mult)
                    # reduce to tmp, then += into A (m-half slice)
                    t1h = smallp.tile([P, MH, H], f32, tag="t1h")
                    nc.vector.tensor_reduce(
                        out=t1h[:],
                        in_=kg[:].rearrange("p m (g d) -> p (m g) d", d=D),
                        axis=mybir.AxisListType.X, op=add)
                    nc.vector.tensor_tensor(
                        out=A[:, :, mh * MH:(mh + 1) * MH],
                        in0=A[:, :, mh * MH:(mh + 1) * MH],
                        in1=t1h[:].transpose([0, 2, 1]), op=add)

                # softmax over m (scale folded into exp)
                E = smallp.tile([P, H, M], f32, tag="E")
                nc.scalar.activation(out=E[:], in_=A[:],
                                     func=mybir.ActivationFunctionType.Exp,
                                     scale=INV_SQRT_D)
                Z = smallp.tile([P, H], f32, tag="Z")
                nc.vector.tensor_reduce(out=Z[:], in_=E[:],
                                        axis=mybir.AxisListType.X, op=add)
                rz = smallp.tile([P, H], f32, tag="rz")
                nc.vector.reciprocal(rz[:], Z[:])
                nc.vector.tensor_tensor(
                    out=E[:], in0=E[:],
                    in1=rz[:][:, :, None].broadcast_to([P, H, M]), op=mult)

                # aggregation per m-half
                agg = smallp.tile([P, C], f32, tag="agg")
                for mh in range(2):
                    vg = gathp.tile([P, MH, C], f32, tag="gbuf")
                    for k in range(NH // NIC):
                        nc.gpsimd.dma_gather(
                            out_ap=vg[:, k * (NIC // P):(k + 1) * (NIC // P), :],
                            in_ap=vdr[:],
                            idxs_ap=idxw_slice(idxw, qb, mh * (NH // NIC) + k,
                                               NIC),
                            num_idxs=NIC, num_idxs_reg=NIC, elem_size=C)
                    nc.vector.tensor_tensor(
                        out=vg[:].rearrange("p m (h d) -> p m h d", d=D),
                        in0=vg[:].rearrange("p m (h d) -> p m h d", d=D),
                        in1=E[:, :, mh * MH:(mh + 1) * MH]
                            .transpose([0, 2, 1])[:, :, :, None]
                            .broadcast_to([P, MH, H, D]),
                        op=mult)
                    half = MH // 2
                    while half >= 1:
                        nc.vector.tensor_tensor(
                            out=vg[:, 0:half, :], in0=vg[:, 0:half, :],
                            in1=vg[:, half:2 * half, :], op=add)
                        half //= 2
                    if mh == 0:
                        nc.vector.tensor_tensor(out=agg[:], in0=vg[:, 0, :],
                                                in1=binmat[:], op=add)
                    else:
                        nc.vector.tensor_tensor(out=agg[:], in0=agg[:],
                                                in1=vg[:, 0, :], op=add)

                # out = agg @ W_out^T + b_out (transpose agg via PE)
                aggT = smallp.tile([P, 2, P], f32, tag="aggT")
                for cc in range(2):
                    pst = psump.tile([P, P], f32, tag="pst")
                    nc.tensor.transpose(pst[:], agg[:, cc * P:(cc + 1) * P],
                                        ident[:])
                    nc.vector.tensor_copy(aggT[:, cc, :], pst[:])
                psO = psump.tile([P, C], f32, tag="psO")
                for cc in range(2):
                    nc.tensor.matmul(psO[:], lhsT=aggT[:, cc, :],
                                     rhs=woT[:, cc, :],
                                     start=(cc == 0), stop=(cc == 1))
                out_sb = smallp.tile([P, C], f32, tag="outsb")
                nc.vector.tensor_tensor(out=out_sb[:], in0=psO[:],
                                        in1=boutmat[:], op=add)
                nc.sync.dma_start(d_out[qb], out_sb[:])

    nc.compile()
    return nc


def idxw_slice(idxw_tile, qb, k, nic):
    """Column slice of the wrapped idx tile for gather chunk k (nic idxs)."""
    ncols = nic // 16
    return idxw_tile[:, qb, k * ncols:(k + 1) * ncols]


def _wrap_idx(lst):
    """int16 list -> [128, len/16] wrapped (pos i -> [i%16, i//16]) and
    replicated across the 8 groups of 16 partitions."""
    n = lst.shape[0]
    w = np.empty((P, n // 16), np.int16)
    blk = lst.reshape(n // 16, 16).T  # [16, n/16]
    for g in range(8):
        w[g * 16:(g + 1) * 16, :] = blk
    return w


def make_core_inputs(pairwise_g, coset_functions, nbhd_idx,
                     W_q, b_q, W_k, W_l, u, v, W_in, b_in, W_out, b_out):
    pairwise_g = np.asarray(pairwise_g)
    coset_functions = np.asarray(coset_functions)
    nbhd_idx = np.asarray(nbhd_idx)

    wqT = np.ascontiguousarray(W_q.T.reshape(2, P, C).astype(np.float32))
    wkT = np.ascontiguousarray(W_k.T.reshape(2, P, C).astype(np.float32))
    winT = np.ascontiguousarray(W_in.T.reshape(2, P, C).astype(np.float32))
    woT = np.ascontiguousarray(W_out.T.reshape(2, P, C).astype(np.float32))
    wlBD_full = np.zeros((C, H * POS), np.float32)
    for h in range(H):
        wlBD_full[h * D:(h + 1) * D, h * POS:(h + 1) * POS] = \
            W_l[h * D:(h + 1) * D, :]
    wlBD = np.ascontiguousarray(wlBD_full.reshape(2, P, H * POS))
    bqv = np.ascontiguousarray(
        (b_q.astype(np.float32) + v.reshape(C).astype(np.float32))
        .reshape(2, P, 1))
    bqmat = np.ascontiguousarray(
        np.broadcast_to(b_q.astype(np.float32), (P, C)))
    binmat = np.ascontiguousarray(
        np.broadcast_to(b_in.astype(np.float32), (P, C)))
    boutmat = np.ascontiguousarray(
        np.broadcast_to(b_out.astype(np.float32), (P, C)))

    in_maps = []
    for core in range(NCORES):
        b = core // 4
        qs = (core % 4) * NQ
        cosetT = np.ascontiguousarray(
            coset_functions[b].T.reshape(2, P, N).astype(np.float32))
        cosetQT = np.ascontiguousarray(
            coset_functions[b, qs:qs + NQ].T.reshape(2, P, NQ)
            .astype(np.float32))
        idx = nbhd_idx[b, qs:qs + NQ].astype(np.int64)  # [NQ, M]

        idxw = np.empty((P, QB, M * P // 16), np.int16)
        pgidxw = np.empty((P, QB, M * P // 16), np.int16)
        pgmask = np.zeros((P, 7, QB, M), np.uint8)
        for qb in range(QB):
            blk = idx[qb * P:(qb + 1) * P]  # [P(n), M]
            # m-major list: pos i = m*128 + n
            lst = blk.T.reshape(M * P)  # [m, n] flattened
            idxw[:, qb, :] = _wrap_idx(lst.astype(np.int16))
            flat = (np.arange(P, dtype=np.int64)[None, :] * N
                    + blk.T)  # [m, n] local flat
            pgidxw[:, qb, :] = _wrap_idx(
                (flat.reshape(M * P) >> 3).astype(np.int16))
            par = (blk & 7)  # [P(n), M] (n*N is a multiple of 8)
            for k in range(1, 8):
                pgmask[:, k - 1, qb, :] = (par == k).astype(np.uint8)

        # packed pairwise_g: row r = flat rows 8r..8r+7, padded 6->8 floats
        pgs = pairwise_g[b, qs:qs + NQ].reshape(NQ * N, POS).astype(np.float32)
        pgpack = np.zeros((NQ * N // 8, 8, 8), np.float32)
        pgpack[:, :, 0:POS] = pgs.reshape(NQ * N // 8, 8, POS)
        pgpack = np.ascontiguousarray(pgpack.reshape(NQ * N // 8, 64))

        in_maps.append({
            "cosetT": cosetT, "cosetQT": cosetQT,
            "wqT": wqT, "wkT": wkT, "winT": winT, "woT": woT,
            "wlBD": wlBD, "bqv": bqv, "bqmat": bqmat,
            "binmat": binmat, "boutmat": boutmat,
            "idxw": idxw, "pgidxw": pgidxw, "pgmask": pgmask,
            "pgpack": pgpack,
        })
    return in_maps


def assemble_output(results):
    out = np.empty((B, N, C), np.float32)
    for core in range(NCORES):
        b = core // 4
        qs = (core % 4) * NQ
        o = results[core]["out"]  # [QB, P, C]
        out[b, qs:qs + NQ] = o.reshape(NQ, C)
    return out


def kernel(pairwise_g, coset_functions, mask, nbhd_idx,
           W_q, b_q, W_k, b_k, W_l, b_l, u, v,
           W_in, b_in, W_out, b_out, **_unused):
    from concourse.bass_utils import run_bass_kernel_spmd

    if "nc" not in _compiled:
        _compiled["nc"] = build_bass()
    nc = _compiled["nc"]

    in_maps = make_core_inputs(pairwise_g, coset_functions, nbhd_idx,
                               W_q, b_q, W_k, W_l, u, v, W_in, b_in,
                               W_out, b_out)
    res = run_bass_kernel_spmd(nc, in_maps, core_ids=list(range(NCORES)))
    return assemble_output(res.results)

